# revision 2
# baseline (speedup 1.0000x reference)
"""Mixture-of-Experts kernel for Trainium2 (8 NeuronCores).

Strategy (expert-parallel, sparse dispatch):
  - Host computes the tiny gate (x @ Wg + bg, [16384, 8]), takes top-2,
    softmaxes the two logits, and dispatches tokens by expert id.
  - Core e receives: its expert's weights (bf16), the first CAP=4096
    tokens routed to it (transposed, bf16, zero-padded), and per-token
    gate weights.  It computes g * gelu(x @ W1 + b1) @ W2 on device.
  - Host scatter-adds per-expert outputs back into token rows, adds the
    gate-weighted b2 term exactly (out += G @ b2), and computes any
    overflow tokens (expert load > CAP; ~0-3% of pairs) in numpy.

Device kernel (per core), all matmuls bf16 with fp32 PSUM accumulation,
4 super-blocks of 1024 tokens:
  mm1: for each ht (32): one LDWEIGHTS per k-tile serves TWO N=512
       matmuls (token halves A/B into separate psum banks) — halves the
       LDWEIGHTS rate vs one-per-matmul, which measures ~50ns/MM cheaper.
       W1 is streamed from HBM per super-block in pre-swizzled
       [HT, P, KD*128] layout (one 4KB row per partition per ht), so h
       for the full 1024-token block fits in SBUF alongside resident W2.
  act: h[ht] = gelu(psum + b1[ht])  (exact erf GELU, bias per partition)
  mm2: y[cs] = h.T @ W2 (tokens on PSUM partitions, one LDWEIGHTS per
       two N=512 matmuls after dedup)
  dve: y *= g  (per-partition = per-token gate scalar)

A post-pass (sem_strip) drops the per-matmul semaphore increments Tile
attaches, keeping only accumulation-group finals; consumers' waits are
rounded up to the next kept increment (always their group-final, so no
added latency).
"""

import numpy as np
import ml_dtypes

B, M, D, E, TOPK = 4096, 4, 1024, 8, 2
H = 4 * D
N = B * M
P = 128
CAP = 4096            # device token capacity per expert
SB = 1024             # tokens per super-block
NSB = CAP // SB       # 4 super-blocks
KD = D // P           # 8 k-tiles over D
HT = H // P           # 32 h-tiles over H

_BUILD_CACHE = {}


def _build(repeat=1, strip=True):
    """Build + compile the per-core bass program (capacity CAP).

    repeat>1 wraps the super-block loop in a hardware For_i that
    re-executes the body `repeat` times — used by the timing harness.
    """
    key = (repeat, strip)
    if key in _BUILD_CACHE:
        return _BUILD_CACHE[key]

    import concourse.mybir as mybir
    import concourse.tile as tile
    from concourse import bacc

    BF = mybir.dt.bfloat16
    F32 = mybir.dt.float32
    GELU = mybir.ActivationFunctionType.Gelu

    nc = bacc.Bacc(trn_type="TRN2", target_bir_lowering=False, debug=False)

    xT = nc.dram_tensor("xT", [KD, P, CAP], BF, kind="ExternalInput")
    w1r = nc.dram_tensor("w1r", [HT, P, KD * P], BF, kind="ExternalInput")
    w2 = nc.dram_tensor("w2", [HT, P, D], BF, kind="ExternalInput")
    b1t = nc.dram_tensor("b1t", [P, HT], F32, kind="ExternalInput")
    gt = nc.dram_tensor("gt", [P, CAP // P], F32, kind="ExternalInput")
    y = nc.dram_tensor("y", [CAP, D], F32, kind="ExternalOutput")

    y_r = y.rearrange("(ncs p) d -> ncs p d", p=P)

    with tile.TileContext(nc) as tc:
        with (
            tc.tile_pool(name="weights", bufs=1) as wp,
            tc.tile_pool(name="w1s", bufs=4) as w1p,
            tc.tile_pool(name="xin", bufs=2) as xp,
            tc.tile_pool(name="hbuf", bufs=1) as hp,
            tc.tile_pool(name="yout", bufs=4) as yp,
            tc.tile_pool(name="ps_a", bufs=2, space="PSUM") as pha,
            tc.tile_pool(name="ps_b", bufs=2, space="PSUM") as phb,
            tc.tile_pool(name="ps_o", bufs=2, space="PSUM") as po,
        ):
            # prologue: biases/gates first (tiny), W2 resident (needed
            # ~270us in, hidden under super-block 0 mm1).
            b1sb = wp.tile([P, HT], F32, tag="b1t", name="b1sb")
            nc.sync.dma_start(b1sb, b1t.ap())
            gtsb = wp.tile([P, CAP // P], F32, tag="gt", name="gtsb")
            nc.sync.dma_start(gtsb, gt.ap())
            w2sb = []
            for ht in range(HT):
                t = wp.tile([P, D], BF, tag=f"w2_{ht}", name=f"w2_{ht}")
                nc.sync.dma_start(t, w2[ht])
                w2sb.append(t)

            import contextlib
            loop_ctx = (
                tc.For_i(0, repeat, 1) if repeat > 1 else contextlib.nullcontext()
            )
            with loop_ctx:
              for b in range(NSB):
                c0 = b * SB
                xblk = []
                for k in range(KD):
                    t = xp.tile([P, SB], BF, tag=f"x{k}", name=f"x{k}")
                    nc.scalar.dma_start(t, xT[k][:, c0:c0 + SB])
                    xblk.append(t)

                h_all = hp.tile([P, HT, SB], BF, tag="h", name="h_all")

                # mm1 + gelu: one LDW per (ht, k) serves both token halves
                for ht in range(HT):
                    w1t = w1p.tile([P, KD * P], BF, tag="w1ht", name="w1t")
                    nc.sync.dma_start(w1t, w1r[ht])
                    psa = pha.tile([P, 512], F32, tag="pa", name="psa")
                    psb = phb.tile([P, 512], F32, tag="pb", name="psb")
                    for k in range(KD):
                        stat = w1t[:, k * P:(k + 1) * P]
                        nc.tensor.matmul(
                            psa, stat, xblk[k][:, 0:512],
                            start=(k == 0), stop=(k == KD - 1),
                        )
                        nc.tensor.matmul(
                            psb, stat, xblk[k][:, 512:SB],
                            start=(k == 0), stop=(k == KD - 1),
                        )
                    nc.scalar.activation(
                        h_all[:, ht, 0:512], psa, GELU, bias=b1sb[:, ht:ht + 1]
                    )
                    nc.scalar.activation(
                        h_all[:, ht, 512:SB], psb, GELU, bias=b1sb[:, ht:ht + 1]
                    )

                # mm2 + gate scale: y[cs] = g * (h.T @ W2); dt innermost so
                # the duplicate LDWEIGHTS is stripped by _dedup_ldweights.
                for cs in range(SB // P):
                    pots = [po.tile([P, 512], F32, tag=f"po{dt}", name=f"po{dt}")
                            for dt in range(2)]
                    for ht in range(HT):
                        for dt in range(2):
                            nc.tensor.matmul(
                                pots[dt],
                                h_all[:, ht, cs * P:(cs + 1) * P],
                                w2sb[ht][:, dt * 512:(dt + 1) * 512],
                                start=(ht == 0), stop=(ht == HT - 1),
                            )
                    gi = c0 // P + cs
                    for dt in range(2):
                        ysb = yp.tile([P, 512], F32, tag=f"y{dt}", name=f"ysb{dt}")
                        nc.vector.tensor_scalar_mul(
                            ysb, pots[dt], gtsb[:, gi:gi + 1]
                        )
                        nc.sync.dma_start(
                            y_r[gi][:, dt * 512:(dt + 1) * 512], ysb
                        )
    _dedup_ldweights(nc)
    if strip:
        from sem_strip import strip_mm_sem_incs
        strip_mm_sem_incs(nc)
    nc.compile()
    _BUILD_CACHE[key] = nc
    return nc


def _ap_key(arg):
    """Stable identity key for an instruction AP argument, or None."""
    try:
        ap = arg.bass_ap if hasattr(arg, "bass_ap") else arg
        t = ap.tensor
        return (t.name, ap.offset, tuple(map(tuple, ap.ap)))
    except Exception:
        return None


def _dedup_ldweights(nc):
    """Drop an InstLdweights when the immediately-preceding PE instruction
    sequence already loaded the identical weights AP (PE weight state is
    sticky until the next LDWEIGHTS). Only sync-free duplicates are dropped.
    """
    import concourse.mybir as mybir

    n_del = 0
    for blk in nc.m.functions[0].blocks:
        insts = list(blk.instructions)
        keep = []
        last_key = None
        for inst in insts:
            tn = type(inst).__name__
            if tn == "InstLdweights":
                key = _ap_key(inst.ins[0])
                si = inst.sync_info
                clean = not (si and (si.on_wait or si.on_update))
                if key is not None and key == last_key and clean:
                    n_del += 1
                    continue
                last_key = key
            elif tn != "InstMatmult" and getattr(inst, "engine", None) == mybir.EngineType.PE:
                last_key = None
            keep.append(inst)
        if len(keep) != len(insts):
            while len(blk.instructions):
                blk.instructions.pop()
            for inst in keep:
                blk.instructions.append(inst)
    return n_del


def _route(xf, Wg, bg):
    """Top-2 gating on host. Returns (idx, gate) per expert and dense G."""
    logits = xf @ Wg + bg                      # [N, E] f32
    n = logits.shape[0]
    ar = np.arange(n)
    i1 = np.argmax(logits, axis=1)
    v1 = logits[ar, i1]
    masked = logits.copy()
    masked[ar, i1] = -np.inf
    i2 = np.argmax(masked, axis=1)
    v2 = masked[ar, i2]
    e2 = np.exp(v2 - v1)
    wt1 = 1.0 / (1.0 + e2)
    wt2 = e2 / (1.0 + e2)
    G = np.zeros_like(logits)
    G[ar, i1] = wt1
    G[ar, i2] = wt2
    idxs, gates = [], []
    for e in range(E):
        idx = np.nonzero((i1 == e) | (i2 == e))[0]
        idxs.append(idx)
        gates.append(G[idx, e].astype(np.float32))
    return idxs, gates, G.astype(np.float32)


def _gelu_exact(x):
    from scipy.special import erf
    return 0.5 * x * (1.0 + erf(x / np.sqrt(2.0)))


def make_in_maps(inputs):
    """Route on host, build per-core device input maps (capacity CAP).

    Returns (in_maps, idxs, gates, G, overflow) where overflow is a list
    of (expert, token_indices, gate_values) for tokens beyond CAP.
    """
    x = np.asarray(inputs["x"], dtype=np.float32)
    Wg = np.asarray(inputs["Wg"], dtype=np.float32)
    bg = np.asarray(inputs["bg"], dtype=np.float32)
    W1 = np.asarray(inputs["W1"], dtype=np.float32)
    b1 = np.asarray(inputs["b1"], dtype=np.float32)
    W2 = np.asarray(inputs["W2"], dtype=np.float32)

    xf = x.reshape(-1, x.shape[-1])
    idxs, gates, G = _route(xf, Wg, bg)

    bf16 = ml_dtypes.bfloat16
    xf_bf = xf.astype(bf16)

    in_maps = []
    overflow = []
    dev_idxs = []
    for e in range(E):
        idx_all, g_all = idxs[e], gates[e]
        ne = min(len(idx_all), CAP)
        if len(idx_all) > CAP:
            overflow.append((e, idx_all[CAP:], g_all[CAP:]))
        idx = idx_all[:ne]
        dev_idxs.append(idx)
        xTe = np.zeros((D, CAP), dtype=bf16)
        xTe[:, :ne] = xf_bf[idx].T
        ge = np.zeros((CAP,), dtype=np.float32)
        ge[:ne] = g_all[:ne]
        w1r = np.ascontiguousarray(
            W1[e].astype(bf16).reshape(KD, P, HT, P).transpose(2, 1, 0, 3)
            .reshape(HT, P, KD * P)
        )
        in_maps.append({
            "xT": np.ascontiguousarray(xTe.reshape(KD, P, CAP)),
            "w1r": w1r,
            "w2": np.ascontiguousarray(W2[e].astype(bf16).reshape(HT, P, D)),
            "b1t": np.ascontiguousarray(b1[e].reshape(HT, P).T),
            "gt": np.ascontiguousarray(ge.reshape(CAP // P, P).T),
        })
    return in_maps, dev_idxs, G, overflow


def kernel(_trace=False, **inputs):
    x = np.asarray(inputs["x"], dtype=np.float32)
    W1 = np.asarray(inputs["W1"], dtype=np.float32)
    b1 = np.asarray(inputs["b1"], dtype=np.float32)
    W2 = np.asarray(inputs["W2"], dtype=np.float32)
    b2 = np.asarray(inputs["b2"], dtype=np.float32)

    Bn, Mn, Dn = x.shape
    xf = x.reshape(-1, Dn)

    in_maps, dev_idxs, G, overflow = make_in_maps(inputs)

    nc = _build()

    from concourse.bass_utils import run_bass_kernel_spmd
    res = run_bass_kernel_spmd(
        nc, in_maps, core_ids=list(range(E)), trace=_trace
    )

    out = G @ b2                               # gate-weighted b2, exact
    for e in range(E):
        idx = dev_idxs[e]
        out[idx] += res.results[e]["y"][:len(idx)]

    # overflow tokens (expert load > CAP) computed exactly on host
    for e, idx, g in overflow:
        h = _gelu_exact(xf[idx] @ W1[e] + b1[e])
        out[idx] += g[:, None] * (h @ W2[e])

    if _trace:
        return out.reshape(Bn, Mn, Dn), res
    return out.reshape(Bn, Mn, Dn)


# revision 4
# speedup vs baseline: 1.8756x; 1.8756x over previous
"""Mixture-of-Experts kernel for Trainium2 (8 NeuronCores).

Strategy (expert-parallel, sparse dispatch):
  - Host computes the tiny gate (x @ Wg + bg, [16384, 8]), takes top-2,
    softmaxes the two logits, and dispatches tokens by expert id.
  - Core e receives: its expert's weights (bf16), the first CAP=4096
    tokens routed to it (transposed, bf16, zero-padded), and per-token
    gate weights.  It computes g * gelu(x @ W1 + b1) @ W2 on device.
  - Host scatter-adds per-expert outputs back into token rows, adds the
    gate-weighted b2 term exactly (out += G @ b2), and computes any
    overflow tokens (expert load > CAP; ~0-3% of pairs) in numpy.

Device kernel (per core), all matmuls bf16 with fp32 PSUM accumulation,
4 super-blocks of 1024 tokens:
  mm1: for each ht (32): one LDWEIGHTS per k-tile serves TWO N=512
       matmuls (token halves A/B into separate psum banks) — halves the
       LDWEIGHTS rate vs one-per-matmul, which measures ~50ns/MM cheaper.
       W1 is streamed from HBM per super-block in pre-swizzled
       [HT, P, KD*128] layout (one 4KB row per partition per ht), so h
       for the full 1024-token block fits in SBUF alongside resident W2.
  act: h[ht] = gelu(psum + b1[ht])  (exact erf GELU, bias per partition)
  mm2: y[cs] = h.T @ W2 (tokens on PSUM partitions, one LDWEIGHTS per
       two N=512 matmuls after dedup)
  dve: y *= g  (per-partition = per-token gate scalar)

A post-pass (sem_strip) drops the per-matmul semaphore increments Tile
attaches, keeping only accumulation-group finals; consumers' waits are
rounded up to the next kept increment (always their group-final, so no
added latency).
"""

import numpy as np
import ml_dtypes

B, M, D, E, TOPK = 4096, 4, 1024, 8, 2
H = 4 * D
N = B * M
P = 128
CAP = 4096            # device token capacity per expert
SB = 1024             # tokens per super-block
NSB = CAP // SB       # 4 super-blocks
KD = D // P           # 8 k-tiles over D
HT = H // P           # 32 h-tiles over H

_BUILD_CACHE = {}


def _build(repeat=1, strip=True):
    """Build + compile the per-core bass program (capacity CAP).

    repeat>1 wraps the super-block loop in a hardware For_i that
    re-executes the body `repeat` times — used by the timing harness.
    """
    key = (repeat, strip)
    if key in _BUILD_CACHE:
        return _BUILD_CACHE[key]

    import concourse.mybir as mybir
    import concourse.tile as tile
    from concourse import bacc

    BF = mybir.dt.bfloat16
    F32 = mybir.dt.float32
    GELU = mybir.ActivationFunctionType.Gelu

    nc = bacc.Bacc(trn_type="TRN2", target_bir_lowering=False, debug=False)

    xT = nc.dram_tensor("xT", [KD, P, CAP], BF, kind="ExternalInput")
    w1r = nc.dram_tensor("w1r", [HT, P, KD * P], BF, kind="ExternalInput")
    w2 = nc.dram_tensor("w2", [HT, P, D], BF, kind="ExternalInput")
    b1t = nc.dram_tensor("b1t", [P, HT], F32, kind="ExternalInput")
    gt = nc.dram_tensor("gt", [P, CAP // P], F32, kind="ExternalInput")
    y = nc.dram_tensor("y", [CAP, D], F32, kind="ExternalOutput")

    y_r = y.rearrange("(ncs p) d -> ncs p d", p=P)

    with tile.TileContext(nc) as tc:
        with (
            tc.tile_pool(name="weights", bufs=1) as wp,
            tc.tile_pool(name="w1s", bufs=4) as w1p,
            tc.tile_pool(name="xin", bufs=2) as xp,
            tc.tile_pool(name="hbuf", bufs=1) as hp,
            tc.tile_pool(name="yout", bufs=4) as yp,
            tc.tile_pool(name="ps_a", bufs=2, space="PSUM") as pha,
            tc.tile_pool(name="ps_b", bufs=2, space="PSUM") as phb,
            tc.tile_pool(name="ps_o", bufs=2, space="PSUM") as po,
        ):
            # prologue: biases/gates first (tiny), W2 resident (needed
            # ~270us in, hidden under super-block 0 mm1).
            b1sb = wp.tile([P, HT], F32, tag="b1t", name="b1sb")
            nc.gpsimd.dma_start(b1sb, b1t.ap())
            gtsb = wp.tile([P, CAP // P], F32, tag="gt", name="gtsb")
            nc.gpsimd.dma_start(gtsb, gt.ap())
            w2sb = []
            for ht in range(HT):
                t = wp.tile([P, D], BF, tag=f"w2_{ht}", name=f"w2_{ht}")
                nc.gpsimd.dma_start(t, w2[ht])
                w2sb.append(t)
            # x for super-block 0 (scalar ring; inside the loop, block b
            # prefetches block (b+1) %% NSB so x DMAs sit ahead of y DMAs
            # in the scalar FIFO ring)
            xtiles = {}
            def load_x(bb):
                lst = []
                for k in range(KD):
                    t = xp.tile([P, SB], BF, tag=f"x{k}", name=f"x{k}")
                    nc.scalar.dma_start(t, xT[k][:, (bb % NSB) * SB:(bb % NSB) * SB + SB])
                    lst.append(t)
                xtiles[bb % NSB] = lst
            load_x(0)

            import contextlib
            loop_ctx = (
                tc.For_i(0, repeat, 1) if repeat > 1 else contextlib.nullcontext()
            )
            with loop_ctx:
              for b in range(NSB):
                c0 = b * SB
                xblk = xtiles[b]
                load_x(b + 1)

                h_all = hp.tile([P, HT, SB], BF, tag="h", name="h_all")

                # mm1 + gelu: one LDW per (ht, k) serves both token halves
                for ht in range(HT):
                    w1t = w1p.tile([P, KD * P], BF, tag="w1ht", name="w1t")
                    nc.sync.dma_start(w1t, w1r[ht])
                    psa = pha.tile([P, 512], F32, tag="pa", name="psa")
                    psb = phb.tile([P, 512], F32, tag="pb", name="psb")
                    for k in range(KD):
                        stat = w1t[:, k * P:(k + 1) * P]
                        nc.tensor.matmul(
                            psa, stat, xblk[k][:, 0:512],
                            start=(k == 0), stop=(k == KD - 1),
                        )
                        nc.tensor.matmul(
                            psb, stat, xblk[k][:, 512:SB],
                            start=(k == 0), stop=(k == KD - 1),
                        )
                    nc.scalar.activation(
                        h_all[:, ht, 0:512], psa, GELU, bias=b1sb[:, ht:ht + 1]
                    )
                    nc.scalar.activation(
                        h_all[:, ht, 512:SB], psb, GELU, bias=b1sb[:, ht:ht + 1]
                    )

                # mm2 + gate scale: y[cs] = g * (h.T @ W2); dt innermost so
                # the duplicate LDWEIGHTS is stripped by _dedup_ldweights.
                for cs in range(SB // P):
                    pots = [po.tile([P, 512], F32, tag=f"po{dt}", name=f"po{dt}")
                            for dt in range(2)]
                    for ht in range(HT):
                        for dt in range(2):
                            nc.tensor.matmul(
                                pots[dt],
                                h_all[:, ht, cs * P:(cs + 1) * P],
                                w2sb[ht][:, dt * 512:(dt + 1) * 512],
                                start=(ht == 0), stop=(ht == HT - 1),
                            )
                    gi = c0 // P + cs
                    for dt in range(2):
                        ysb = yp.tile([P, 512], F32, tag=f"y{dt}", name=f"ysb{dt}")
                        nc.vector.tensor_scalar_mul(
                            ysb, pots[dt], gtsb[:, gi:gi + 1]
                        )
                        nc.gpsimd.dma_start(
                            y_r[gi][:, dt * 512:(dt + 1) * 512], ysb
                        )
    _dedup_ldweights(nc)
    if strip:
        from sem_strip import strip_mm_sem_incs
        strip_mm_sem_incs(nc)
    nc.compile()
    _BUILD_CACHE[key] = nc
    return nc


def _ap_key(arg):
    """Stable identity key for an instruction AP argument, or None."""
    try:
        ap = arg.bass_ap if hasattr(arg, "bass_ap") else arg
        t = ap.tensor
        return (t.name, ap.offset, tuple(map(tuple, ap.ap)))
    except Exception:
        return None


def _dedup_ldweights(nc):
    """Drop an InstLdweights when the immediately-preceding PE instruction
    sequence already loaded the identical weights AP (PE weight state is
    sticky until the next LDWEIGHTS). Only sync-free duplicates are dropped.
    """
    import concourse.mybir as mybir

    n_del = 0
    for blk in nc.m.functions[0].blocks:
        insts = list(blk.instructions)
        keep = []
        last_key = None
        for inst in insts:
            tn = type(inst).__name__
            if tn == "InstLdweights":
                key = _ap_key(inst.ins[0])
                si = inst.sync_info
                clean = not (si and (si.on_wait or si.on_update))
                if key is not None and key == last_key and clean:
                    n_del += 1
                    continue
                last_key = key
            elif tn != "InstMatmult" and getattr(inst, "engine", None) == mybir.EngineType.PE:
                last_key = None
            keep.append(inst)
        if len(keep) != len(insts):
            while len(blk.instructions):
                blk.instructions.pop()
            for inst in keep:
                blk.instructions.append(inst)
    return n_del


def _route(xf, Wg, bg):
    """Top-2 gating on host. Returns (idx, gate) per expert and dense G."""
    logits = xf @ Wg + bg                      # [N, E] f32
    n = logits.shape[0]
    ar = np.arange(n)
    i1 = np.argmax(logits, axis=1)
    v1 = logits[ar, i1]
    masked = logits.copy()
    masked[ar, i1] = -np.inf
    i2 = np.argmax(masked, axis=1)
    v2 = masked[ar, i2]
    e2 = np.exp(v2 - v1)
    wt1 = 1.0 / (1.0 + e2)
    wt2 = e2 / (1.0 + e2)
    G = np.zeros_like(logits)
    G[ar, i1] = wt1
    G[ar, i2] = wt2
    idxs, gates = [], []
    for e in range(E):
        idx = np.nonzero((i1 == e) | (i2 == e))[0]
        idxs.append(idx)
        gates.append(G[idx, e].astype(np.float32))
    return idxs, gates, G.astype(np.float32)


def _gelu_exact(x):
    from scipy.special import erf
    return 0.5 * x * (1.0 + erf(x / np.sqrt(2.0)))


def make_in_maps(inputs):
    """Route on host, build per-core device input maps (capacity CAP).

    Returns (in_maps, idxs, gates, G, overflow) where overflow is a list
    of (expert, token_indices, gate_values) for tokens beyond CAP.
    """
    x = np.asarray(inputs["x"], dtype=np.float32)
    Wg = np.asarray(inputs["Wg"], dtype=np.float32)
    bg = np.asarray(inputs["bg"], dtype=np.float32)
    W1 = np.asarray(inputs["W1"], dtype=np.float32)
    b1 = np.asarray(inputs["b1"], dtype=np.float32)
    W2 = np.asarray(inputs["W2"], dtype=np.float32)

    xf = x.reshape(-1, x.shape[-1])
    idxs, gates, G = _route(xf, Wg, bg)

    bf16 = ml_dtypes.bfloat16
    xf_bf = xf.astype(bf16)

    in_maps = []
    overflow = []
    dev_idxs = []
    for e in range(E):
        idx_all, g_all = idxs[e], gates[e]
        ne = min(len(idx_all), CAP)
        if len(idx_all) > CAP:
            overflow.append((e, idx_all[CAP:], g_all[CAP:]))
        idx = idx_all[:ne]
        dev_idxs.append(idx)
        xTe = np.zeros((D, CAP), dtype=bf16)
        xTe[:, :ne] = xf_bf[idx].T
        ge = np.zeros((CAP,), dtype=np.float32)
        ge[:ne] = g_all[:ne]
        w1r = np.ascontiguousarray(
            W1[e].astype(bf16).reshape(KD, P, HT, P).transpose(2, 1, 0, 3)
            .reshape(HT, P, KD * P)
        )
        in_maps.append({
            "xT": np.ascontiguousarray(xTe.reshape(KD, P, CAP)),
            "w1r": w1r,
            "w2": np.ascontiguousarray(W2[e].astype(bf16).reshape(HT, P, D)),
            "b1t": np.ascontiguousarray(b1[e].reshape(HT, P).T),
            "gt": np.ascontiguousarray(ge.reshape(CAP // P, P).T),
        })
    return in_maps, dev_idxs, G, overflow


def kernel(_trace=False, **inputs):
    x = np.asarray(inputs["x"], dtype=np.float32)
    W1 = np.asarray(inputs["W1"], dtype=np.float32)
    b1 = np.asarray(inputs["b1"], dtype=np.float32)
    W2 = np.asarray(inputs["W2"], dtype=np.float32)
    b2 = np.asarray(inputs["b2"], dtype=np.float32)

    Bn, Mn, Dn = x.shape
    xf = x.reshape(-1, Dn)

    in_maps, dev_idxs, G, overflow = make_in_maps(inputs)

    nc = _build()

    from concourse.bass_utils import run_bass_kernel_spmd
    res = run_bass_kernel_spmd(
        nc, in_maps, core_ids=list(range(E)), trace=_trace
    )

    out = G @ b2                               # gate-weighted b2, exact
    for e in range(E):
        idx = dev_idxs[e]
        out[idx] += res.results[e]["y"][:len(idx)]

    # overflow tokens (expert load > CAP) computed exactly on host
    for e, idx, g in overflow:
        h = _gelu_exact(xf[idx] @ W1[e] + b1[e])
        out[idx] += g[:, None] * (h @ W2[e])

    if _trace:
        return out.reshape(Bn, Mn, Dn), res
    return out.reshape(Bn, Mn, Dn)
